# revision 3
# baseline (speedup 1.0000x reference)
"""ChebConvolution (K=4) Trainium2 kernel, 8-way sharded.

Math: with P = spmm(2*adj_vals) and right-multiplication by W commuting
with the (linear) sparse propagation, the reference collapses to

    Y = P(P X) W^3 - (P X) W^3 - X W^2
      = P(table2) - V,   table2 = Z1 @ W^3,  Z1 = P X,
                         V = Z1 @ W^3 + X @ W^2

Per core c (rows [c*S, (c+1)*S)):
  phase 1: SpMM Z1 rows via dma_gather from replicated bf16 X table +
           one-hot*val mask matmuls accumulated in PSUM (feature-major),
           then t2 = Z1 @ W3 and V = t2 + X @ W2 per 128-row block.
  AllGather t2 shards -> full bf16 table2 in every core's HBM.
  phase 2: SpMM P(table2) rows (node-major psum), subtract V, write Y shard.

The mask trick: for a tile of 128 edges, M[e, d] = (IOTA[e,d] == dest[e]) *
val[e] is ONE DVE tensor_scalar (is_equal, mult) with per-partition scalars;
psum += M.T @ G (or G.T @ M) does scale + segment-sum on the PE. Edges are
sorted by dest; a 128-row dest block maps to a (cross-core union) range of
tiles, and boundary tiles are simply multiplied into both adjacent blocks'
psums - out-of-block dests never match the IOTA so they contribute zero.
This avoids all per-block gather padding: Q7 SWDGE descriptor generation
(~8ns/idx) is the bottleneck, so gathered-edge count is minimized while the
(cheap) DVE mask builds absorb the schedule overlap.

Edges are partitioned by dest core and split by source half (int16 gather
index limit); per-(core,half) streams are padded only at the end to a
cross-core-uniform tile count so one NEFF serves all 8 cores.
"""

import os
import sys

for _p in ("/opt/trn_rl_repo", "/root/.axon_site/_ro/trn_rl_repo"):
    if os.path.isdir(_p) and _p not in sys.path:
        sys.path.insert(0, _p)

import numpy as np
import ml_dtypes

import concourse.bacc as bacc
import concourse.mybir as mybir
import concourse.tile as tile
from concourse.bass_utils import run_bass_kernel_spmd

F32 = mybir.dt.float32
BF16 = mybir.dt.bfloat16
I16 = mybir.dt.int16

D = 128            # feature dim (in == out == 128)
SPLIT = 32768      # int16 gather index limit -> lo/hi table halves
CH_TILES = 8       # gather chunk: 1024 idx (HW dma_gather limit)
PAD_DEST = 3.0e8   # dest sentinel for padding edges (never matches IOTA)


def _pack_idxs(flat_idx):
    """int16 gather index layout: [128, n/16], idx j at [16k + j%16, j//16]."""
    n = len(flat_idx)
    assert n % 16 == 0
    arr = flat_idx.astype(np.int16).reshape(n // 16, 16).T  # [16, n/16]
    return np.tile(arr, (8, 1))


def _host_prep(N, ncores, adj_rows, adj_cols, adj_vals):
    """Sort/pad edges into per-core uniform tile streams + union schedule.

    Returns sched (cross-core constants incl. per-block mm lists) and
    per-core input arrays.
    """
    S = N // ncores
    NB = (S + 127) // 128
    rows = adj_rows.astype(np.int64)
    cols = adj_cols.astype(np.int64)
    vals2 = (2.0 * adj_vals).astype(np.float32)

    core = rows // S
    dloc = rows - core * S
    half = (cols >= SPLIT).astype(np.int64)

    # per-(core, half) edge counts -> uniform padded tile counts
    ch_key = core * 2 + half
    cnt = np.bincount(ch_key, minlength=ncores * 2).reshape(ncores, 2)
    T_half = [max(int(-(-cnt[:, h].max() // 128)), 1) for h in (0, 1)]
    if N <= SPLIT:
        T_half[1] = 0

    # order edges by (core, half, dest, col); rank within (core, half)
    order = np.lexsort((cols, dloc, half, core))
    k_s = ch_key[order]
    firsts = np.r_[0, np.flatnonzero(np.diff(k_s)) + 1]
    seg_of = np.cumsum(np.isin(np.arange(len(k_s)), firsts)) - 1
    rank = np.arange(len(k_s)) - firsts[seg_of]

    core_s, half_s = core[order], half[order]
    col_s, dloc_s, val_s = cols[order], dloc[order], vals2[order]
    tile_in_half = rank // 128

    # union tile range per (half, block): tiles any core's block-b edges touch
    blk_s = dloc_s // 128
    NMM_ranges = {}
    for h in range(2):
        if T_half[h] == 0:
            continue
        m = half_s == h
        # min/max tile per (core, block)
        kb = core_s[m] * NB + blk_s[m]
        tmin = np.full(ncores * NB, 1 << 30, np.int64)
        tmax = np.full(ncores * NB, -1, np.int64)
        np.minimum.at(tmin, kb, tile_in_half[m])
        np.maximum.at(tmax, kb, tile_in_half[m])
        tmin = tmin.reshape(ncores, NB)
        tmax = tmax.reshape(ncores, NB)
        for b in range(NB):
            lo = int(tmin[:, b].min())
            hi = int(tmax[:, b].max())
            if hi < 0:  # no core has (h, b) edges
                NMM_ranges[(h, b)] = (0, 0)
            else:
                NMM_ranges[(h, b)] = (lo, hi + 1)

    # per-block mm list: (half, tile) pairs; ensure >= 1 mm per block
    block_mms = []
    for b in range(NB):
        mms = []
        for h in range(2):
            if T_half[h] == 0:
                continue
            s, e = NMM_ranges.get((h, b), (0, 0))
            for t in range(s, e):
                mms.append((h, t))
        if not mms:
            mms.append((0, 0))
        block_mms.append(mms)
    NMM = sum(len(m) for m in block_mms)

    # global mm index per (block, position)
    mm_index = {}
    i = 0
    for b, mms in enumerate(block_mms):
        for h, t in mms:
            mm_index[(b, h, t)] = i
            i += 1

    # fill per-core data arrays
    idx_streams = [np.zeros((ncores, max(T_half[h], 1) * 128), np.int64)
                   for h in range(2)]
    dest_all = np.full((ncores, NMM * 128), PAD_DEST, np.float32)
    val_all = np.zeros((ncores, NMM * 128), np.float32)

    slot = rank
    for h in range(2):
        m = half_s == h
        if not m.any():
            continue
        idx_streams[h][core_s[m], slot[m]] = col_s[m] - (SPLIT if h else 0)

    # Each edge writes dest/val into the ONE mm of (its block, its tile);
    # other mms covering the same tile keep PAD dest (-> zero mask cols).
    T_max = max(max(T_half), 1)
    mm_lookup = np.full((2, T_max, NB), -1, np.int64)
    for (b, h, t), i in mm_index.items():
        mm_lookup[h, t, b] = i
    i_s = mm_lookup[half_s, tile_in_half, blk_s]
    assert (i_s >= 0).all(), "schedule does not cover some edge"
    edge_part = slot % 128
    gpos = core_s * (NMM * 128) + i_s * 128 + edge_part
    dest_all.reshape(-1)[gpos] = (dloc_s - 128 * blk_s).astype(np.float32)
    val_all.reshape(-1)[gpos] = val_s

    sched = dict(S=S, NB=NB, T_lo=T_half[0], T_hi=T_half[1], NMM=NMM,
                 block_mms=block_mms, mm_index=mm_index)
    per_core = []
    for c in range(ncores):
        per_core.append(dict(
            idx_lo=_pack_idxs(idx_streams[0][c]),
            idx_hi=_pack_idxs(idx_streams[1][c]) if T_half[1] else None,
            dest=np.ascontiguousarray(
                dest_all[c].reshape(NMM, 128).T),
            val=np.ascontiguousarray(val_all[c].reshape(NMM, 128).T),
        ))
    return sched, per_core


def _build_program(N, ncores, sched):
    S, NB, NMM = sched["S"], sched["NB"], sched["NMM"]
    T_lo, T_hi = sched["T_lo"], sched["T_hi"]
    block_mms, mm_index = sched["block_mms"], sched["mm_index"]

    nc = bacc.Bacc("TRN2", target_bir_lowering=False,
                   num_devices=(ncores if ncores > 1 else None))

    tab_d = nc.dram_tensor("tab", [N, D], BF16, kind="ExternalInput")
    xT_d = nc.dram_tensor("xT", [D, NB * 128], BF16, kind="ExternalInput")
    w_d = nc.dram_tensor("w", [D, D], F32, kind="ExternalInput")
    wT_d = nc.dram_tensor("wT", [D, D], F32, kind="ExternalInput")
    iota_d = nc.dram_tensor("iota", [128, 128], BF16, kind="ExternalInput")
    ixlo_d = nc.dram_tensor("ixlo", [128, T_lo * 8], I16, kind="ExternalInput")
    if T_hi:
        ixhi_d = nc.dram_tensor("ixhi", [128, T_hi * 8], I16, kind="ExternalInput")
    dest_d = nc.dram_tensor("dest", [128, NMM], F32, kind="ExternalInput")
    val_d = nc.dram_tensor("val", [128, NMM], F32, kind="ExternalInput")
    y_d = nc.dram_tensor("y", [S, D], F32, kind="ExternalOutput")

    cc_in = nc.dram_tensor("cc_in", [S, D], BF16, kind="Internal")
    cc_out = nc.dram_tensor("cc_out", [N, D], BF16, kind="Internal",
                            addr_space="Shared")

    ixlo_sb = nc.alloc_sbuf_tensor("ixlo_sb", [128, T_lo * 8], I16)
    ixhi_sb = nc.alloc_sbuf_tensor("ixhi_sb", [128, T_hi * 8], I16) if T_hi else None
    dest_sb = nc.alloc_sbuf_tensor("dest_sb", [128, NMM], F32)
    val_sb = nc.alloc_sbuf_tensor("val_sb", [128, NMM], F32)
    iota_sb = nc.alloc_sbuf_tensor("iota_sb", [128, 128], BF16)
    xT_sb = nc.alloc_sbuf_tensor("xT_sb", [D, NB * 128], BF16)
    w_sb = nc.alloc_sbuf_tensor("w_sb", [D, D], F32)
    wT_sb = nc.alloc_sbuf_tensor("wT_sb", [D, D], F32)
    w2_sb = nc.alloc_sbuf_tensor("w2_sb", [D, D], F32)
    w2bf_sb = nc.alloc_sbuf_tensor("w2bf_sb", [D, D], BF16)
    w3bf_sb = nc.alloc_sbuf_tensor("w3bf_sb", [D, D], BF16)
    v_sb = nc.alloc_sbuf_tensor("v_sb", [128, NB * 128], F32)

    n_tiles = (T_lo, T_hi)

    def chunks(tot):
        out = []
        t0 = 0
        while t0 < tot:
            ct = min(CH_TILES, tot - t0)
            out.append((t0, ct))
            t0 += ct
        return out

    stream_chunks = (chunks(T_lo), chunks(T_hi))

    with tile.TileContext(nc) as tc:
        nc.sync.dma_start(ixlo_sb[:], ixlo_d[:])
        if T_hi:
            nc.sync.dma_start(ixhi_sb[:], ixhi_d[:])
        nc.sync.dma_start(dest_sb[:], dest_d[:])
        nc.sync.dma_start(val_sb[:], val_d[:])
        nc.sync.dma_start(iota_sb[:], iota_d[:])
        nc.sync.dma_start(xT_sb[:], xT_d[:])
        nc.sync.dma_start(w_sb[:], w_d[:])
        nc.sync.dma_start(wT_sb[:], wT_d[:])

        with (
            tc.tile_pool(name="wps", bufs=2, space="PSUM") as wps,
            tc.tile_pool(name="wsb", bufs=2) as wsb,
        ):
            w2_ps = wps.tile([D, D], F32, name="w2_ps")
            nc.tensor.matmul(w2_ps[:], wT_sb[:], w_sb[:], start=True, stop=True)
            nc.vector.tensor_copy(w2_sb[:], w2_ps[:])
            nc.vector.tensor_copy(w2bf_sb[:], w2_ps[:])
            w3_ps = wps.tile([D, D], F32, name="w3_ps")
            nc.tensor.matmul(w3_ps[:], wT_sb[:], w2_sb[:], start=True, stop=True)
            nc.vector.tensor_copy(w3bf_sb[:], w3_ps[:])

        def emit_spmm(phase, tab_lo_ap, tab_hi_ap, per_block_tail):
            with (
                tc.tile_pool(name=f"g{phase}", bufs=3) as gpool,
                tc.tile_pool(name=f"m{phase}", bufs=6) as mpool,
                tc.tile_pool(name=f"ps{phase}", bufs=3, space="PSUM") as ppool,
                tc.tile_pool(name=f"tail{phase}", bufs=2, space="PSUM") as tpool,
                tc.tile_pool(name=f"sb{phase}", bufs=3) as spool,
            ):
                gbufs = {}

                def ensure_chunk(h, ci):
                    k = (h, ci)
                    if k in gbufs:
                        return gbufs[k]
                    t0, ct = stream_chunks[h][ci]
                    n = ct * 128
                    g = gpool.tile([128, CH_TILES, 128], BF16,
                                   tag=f"g{h}", name=f"g{phase}_{h}_{ci}")
                    ix = (ixlo_sb, ixhi_sb)[h]
                    tab = (tab_lo_ap, tab_hi_ap)[h]
                    nc.gpsimd.dma_gather(
                        g[:, :ct, :], tab, ix[:, t0 * 8:(t0 + ct) * 8], n, n, D)
                    gbufs[k] = g
                    return g

                for b in range(NB):
                    mms = block_mms[b]
                    ps = ppool.tile([128, 128], F32, tag="ps", name=f"ps{phase}_{b}")
                    for j, (h, t) in enumerate(mms):
                        i = mm_index[(b, h, t)]
                        g = ensure_chunk(h, t // CH_TILES)
                        tic = t % CH_TILES
                        m = mpool.tile([128, 128], BF16, tag="m",
                                       name=f"m{phase}_{b}_{j}")
                        nc.vector.tensor_scalar(
                            m[:], iota_sb[:],
                            dest_sb[:, i:i + 1], val_sb[:, i:i + 1],
                            mybir.AluOpType.is_equal, mybir.AluOpType.mult)
                        first, last = (j == 0), (j == len(mms) - 1)
                        if phase == 1:
                            nc.tensor.matmul(ps[:], g[:, tic, :], m[:],
                                             start=first, stop=last)
                        else:
                            nc.tensor.matmul(ps[:], m[:], g[:, tic, :],
                                             start=first, stop=last)
                    per_block_tail(b, ps, tpool, spool)

        def tail1(b, ps, tpool, spool):
            rows = min(128, S - 128 * b)
            z1t = spool.tile([128, 128], BF16, tag="z1t", name=f"z1t_{b}")
            nc.scalar.copy(z1t[:], ps[:])                      # ACT [f,d] bf16
            t2_ps = tpool.tile([128, 128], F32, tag="t2ps", name=f"t2ps_{b}")
            nc.tensor.matmul(t2_ps[:], z1t[:], w3bf_sb[:], start=True, stop=True)
            u_ps = tpool.tile([128, 128], F32, tag="ups", name=f"ups_{b}")
            nc.tensor.matmul(u_ps[:], xT_sb[:, b * 128:(b + 1) * 128],
                             w2bf_sb[:], start=True, stop=True)
            t2t = spool.tile([128, 128], BF16, tag="t2t", name=f"t2t_{b}")
            nc.scalar.copy(t2t[:], t2_ps[:])                   # ACT f32->bf16
            nc.vector.tensor_tensor(v_sb[:, b * 128:(b + 1) * 128],
                                    u_ps[:], t2t[:], mybir.AluOpType.add)
            nc.sync.dma_start(cc_in[b * 128:b * 128 + rows, :], t2t[:rows, :])

        def tail2(b, ps, tpool, spool):
            rows = min(128, S - 128 * b)
            y = spool.tile([128, 128], F32, tag="y", name=f"y_{b}")
            nc.vector.tensor_tensor(y[:], ps[:], v_sb[:, b * 128:(b + 1) * 128],
                                    mybir.AluOpType.subtract)
            nc.sync.dma_start(y_d[b * 128:b * 128 + rows, :], y[:rows, :])

        hi_rows = N - SPLIT if N > SPLIT else 0
        emit_spmm(1, tab_d[0:min(SPLIT, N), :],
                  tab_d[SPLIT:N, :] if hi_rows else None, tail1)

        if ncores > 1:
            nc.gpsimd.collective_compute(
                "AllGather", mybir.AluOpType.bypass,
                replica_groups=[list(range(ncores))],
                ins=[cc_in[:]], outs=[cc_out[:]])
        else:
            nc.sync.dma_start(cc_out[:], cc_in[:])

        emit_spmm(2, cc_out[0:min(SPLIT, N), :],
                  cc_out[SPLIT:N, :] if hi_rows else None, tail2)

    nc.compile()
    return nc


def _make_in_maps(N, ncores, sched, per_core, input_np, W_np):
    S, NB = sched["S"], sched["NB"]
    tab = input_np.astype(ml_dtypes.bfloat16)
    iota = np.broadcast_to(np.arange(128, dtype=np.float32),
                           (128, 128)).astype(ml_dtypes.bfloat16).copy()
    W = W_np.astype(np.float32)
    WT = np.ascontiguousarray(W.T)
    in_maps = []
    for c in range(ncores):
        xT = np.zeros((D, NB * 128), ml_dtypes.bfloat16)
        xT[:, :S] = tab[c * S:(c + 1) * S].T
        m = dict(tab=tab, xT=xT, w=W, wT=WT, iota=iota,
                 ixlo=per_core[c]["idx_lo"],
                 dest=per_core[c]["dest"], val=per_core[c]["val"])
        if sched["T_hi"]:
            m["ixhi"] = per_core[c]["idx_hi"]
        in_maps.append(m)
    return in_maps


_cache = {}


def _get_program(N, ncores, sched):
    key = (N, ncores, sched["NMM"], sched["T_lo"], sched["T_hi"])
    if key not in _cache:
        _cache[key] = _build_program(N, ncores, sched)
    return _cache[key]


def run(input, adj_rows, adj_cols, adj_vals, W, ncores=8, trace=False):
    N = input.shape[0]
    sched, per_core = _host_prep(N, ncores, adj_rows, adj_cols, adj_vals)
    nc = _get_program(N, ncores, sched)
    in_maps = _make_in_maps(N, ncores, sched, per_core, np.asarray(input),
                            np.asarray(W))
    res = run_bass_kernel_spmd(nc, in_maps, core_ids=list(range(ncores)),
                               trace=trace)
    y = np.concatenate([res.results[c]["y"] for c in range(ncores)], axis=0)
    return y[:N].astype(np.float32), res


def kernel(input, adj_rows, adj_cols, adj_vals, W):
    y, _ = run(np.asarray(input), np.asarray(adj_rows), np.asarray(adj_cols),
               np.asarray(adj_vals), np.asarray(W), ncores=8)
    return y
